# revision 3
# baseline (speedup 1.0000x reference)
"""Bass/Tile Trainium2 kernel for nn_DiffusionTransformerBlock.

kernel(**inputs) takes the FULL unsharded inputs of the reference model
and returns the FULL [B,S,D] output, distributing work across 8
NeuronCores internally (SPMD, no collectives).

Sharding: core = b*4 + c handles batch b, query-token chunk c (512
tokens), recomputing K/V for its batch's full sequence (communication-
free; +33% matmul flops but no collective latency/risk).

Dataflow is fully transposed ([feature, token] layout) so projections
need no on-chip transposes; RoPE runs on deinterleaved head dims via a
host-side column permutation of wq/wk; the rmsnorm scale is folded into
the cos/sin tables (Q,K), a per-partition scale at V eviction, and the
norm weights are folded into projection weights host-side. Matmuls run
in float32r (full PE rate, ~13-bit mantissa).
"""
import sys

sys.path.insert(0, "/opt/trn_rl_repo")

import numpy as np

B, S, D, H = 2, 2048, 2048, 16
HD = D // H          # 128
HID = 5632
EPS = 1e-5
NCORES = 8
CH = S // 4          # 512 own tokens per core
P = 128
NT = S // P          # 16 token tiles
ND = D // P          # 16 feature tiles
NHID = HID // P      # 44
HF = HD // 2         # 64
ISQ = float(1.0 / np.sqrt(HD))

_cache = {}


def _patch_ldw_opt():
    import concourse.bass_utils as bass_utils
    if getattr(bass_utils, "_ldw_opt_patched", False):
        return
    orig = bass_utils.run_command

    def patched(argv, **kwargs):
        argv = [a.replace("--enable-ldw-opt=false", "--enable-ldw-opt=true") for a in argv]
        return orig(argv, **kwargs)

    bass_utils.run_command = patched
    bass_utils._ldw_opt_patched = True


def build_nc():
    import concourse.tile as tile
    from concourse import bacc, mybir

    dt = mybir.dt
    F32, F32R = dt.float32, dt.float32r
    AF = mybir.ActivationFunctionType
    ALU = mybir.AluOpType

    nc = bacc.Bacc("TRN2", target_bir_lowering=False, debug=False, num_devices=NCORES)

    xT = nc.dram_tensor("xT", [D, S], F32R, kind="ExternalInput")
    xT_own = nc.dram_tensor("xT_own", [D, CH], F32R, kind="ExternalInput")
    cosT = nc.dram_tensor("cosT", [HF, S], F32, kind="ExternalInput")
    sinT = nc.dram_tensor("sinT", [HF, S], F32, kind="ExternalInput")
    cosT_own = nc.dram_tensor("cosT_own", [HF, CH], F32, kind="ExternalInput")
    sinT_own = nc.dram_tensor("sinT_own", [HF, CH], F32, kind="ExternalInput")
    wq = nc.dram_tensor("wq", [D, D], F32R, kind="ExternalInput")
    wk = nc.dram_tensor("wk", [D, D], F32R, kind="ExternalInput")
    wv = nc.dram_tensor("wv", [D, D], F32R, kind="ExternalInput")
    wo = nc.dram_tensor("wo", [D, D], F32R, kind="ExternalInput")
    w1 = nc.dram_tensor("w1", [D, HID], F32R, kind="ExternalInput")
    w3 = nc.dram_tensor("w3", [D, HID], F32R, kind="ExternalInput")
    w2 = nc.dram_tensor("w2", [HID, D], F32R, kind="ExternalInput")
    yT = nc.dram_tensor("yT", [D, CH], F32, kind="ExternalOutput")

    def rope_evict(rope_p, ps_in, out_ap, cos1, sin1):
        t1 = rope_p.tile([HF, cos1.shape[-1]], F32, tag="t1", name="t1")
        t2 = rope_p.tile([HF, cos1.shape[-1]], F32, tag="t2", name="t2")
        t3 = rope_p.tile([HF, cos1.shape[-1]], F32, tag="t3", name="t3")
        t4 = rope_p.tile([HF, cos1.shape[-1]], F32, tag="t4", name="t4")
        a = ps_in[0:HF, :]
        b = ps_in[HF:HD, :]
        nc.vector.tensor_tensor(t1[:], a, cos1, ALU.mult)
        nc.vector.tensor_tensor(t2[:], b, sin1, ALU.mult)
        nc.vector.tensor_tensor(t3[:], a, sin1, ALU.mult)
        nc.vector.tensor_tensor(t4[:], b, cos1, ALU.mult)
        nc.vector.tensor_tensor(out_ap[0:HF, :], t1[:], t2[:], ALU.subtract)
        nc.vector.tensor_tensor(out_ap[HF:HD, :], t3[:], t4[:], ALU.add)

    def rms_row(pool, pspool, src3d, n, ones_c, eps_c):
        """1/sqrt(mean_d(src^2)+eps) along partitions+tiles -> [1, n] f32 tile."""
        ps_ms = pspool.tile([1, n], F32, tag="msrow", name="ps_ms")
        for d in range(ND):
            sq = pool.tile([P, n], F32R, tag="sq", name="sq")
            nc.scalar.activation(sq[:], src3d[:, d, :], AF.Square)
            nc.tensor.matmul(ps_ms[:], ones_c[:, 0:1], sq[:],
                             start=(d == 0), stop=(d == ND - 1))
        rms = pool.tile([1, n], F32, tag="rms", name="rms")
        nc.scalar.activation(rms[:], ps_ms[:], AF.Sqrt, bias=eps_c[0:1, :], scale=1.0 / D)
        srow = pool.tile([1, n], F32, tag="srow", name="srow")
        nc.vector.reciprocal(srow[:], rms[:])
        return srow

    with tile.TileContext(nc) as tc:
        dram_cm = tc.tile_pool(name="dram", bufs=1, space="DRAM")
        dram = dram_cm.__enter__()
        consts_cm = tc.tile_pool(name="consts", bufs=1)
        consts = consts_cm.__enter__()
        own_cm = tc.tile_pool(name="own", bufs=1)
        own = own_cm.__enter__()

        KT_d = dram.tile([H, HD, S], F32R)
        V_d = dram.tile([S, D], F32R)

        ones_f = consts.tile([P, 2], F32)
        nc.vector.memset(ones_f[:], 1.0)
        ones_c = consts.tile([P, 2], F32R)
        nc.vector.tensor_copy(ones_c[:], ones_f[:])
        eps_c = consts.tile([P, 1], F32)
        nc.vector.memset(eps_c[:], EPS)

        xo = own.tile([P, ND, CH], F32R)
        nc.sync.dma_start(xo[:], xT_own.ap().rearrange("(a p) t -> p a t", p=P))

        # ================= Phase A: full-seq s, K^T -> HBM, V -> HBM =================
        with tc.tile_pool(name="xa", bufs=1) as xa_p, \
             tc.tile_pool(name="sqp", bufs=2) as sq_p, \
             tc.tile_pool(name="wstr", bufs=4) as wstr, \
             tc.tile_pool(name="wvg", bufs=1) as wvg_p, \
             tc.tile_pool(name="rope", bufs=2) as rope_p, \
             tc.tile_pool(name="ev", bufs=3) as ev_p, \
             tc.tile_pool(name="sc", bufs=2) as sc_p, \
             tc.tile_pool(name="psk", bufs=4, space="PSUM") as psk, \
             tc.tile_pool(name="psv", bufs=2, space="PSUM") as psv, \
             tc.tile_pool(name="pss", bufs=1, space="PSUM") as pss:

            for half in range(2):
                t0 = half * (S // 2)
                HS = S // 2
                xa = xa_p.tile([P, ND, HS], F32R, tag="xa", name="xa")
                nc.sync.dma_start(
                    xa[:], xT.ap()[:, t0:t0 + HS].rearrange("(a p) t -> p a t", p=P))

                # --- per-512-chunk s_row -> cos1/sin1 (norm scale folded) ---
                cos1 = sc_p.tile([HF, HS], F32, tag="cos1", name="cos1")
                sin1 = sc_p.tile([HF, HS], F32, tag="sin1", name="sin1")
                for c in range(HS // CH):
                    sl = slice(c * CH, (c + 1) * CH)
                    srow = rms_row(sq_p, pss, xa[:, :, sl], CH, ones_c, eps_c)
                    sb = sc_p.tile([HF, CH], F32, tag="sb", name="sb")
                    nc.gpsimd.partition_broadcast(sb[:], srow[:])
                    ct = sq_p.tile([HF, CH], F32, tag="ct", name="ct")
                    nc.sync.dma_start(ct[:], cosT[:, t0 + c * CH:t0 + (c + 1) * CH])
                    st = sq_p.tile([HF, CH], F32, tag="st", name="st")
                    nc.sync.dma_start(st[:], sinT[:, t0 + c * CH:t0 + (c + 1) * CH])
                    nc.vector.tensor_tensor(cos1[:, sl], ct[:], sb[:], ALU.mult)
                    nc.vector.tensor_tensor(sin1[:, sl], st[:], sb[:], ALU.mult)

                # --- s_col (per-token 1/rms as column) for V scaling ---
                scol_i = sc_p.tile([P, NT // 2], F32, tag="scoli", name="scol_i")
                scol = sc_p.tile([P, NT // 2], F32, tag="scol", name="scol")
                for tt in range(NT // 2):
                    ps_mc = pss.tile([P, 2], F32, tag="mscol", name="ps_mc")
                    for d in range(ND):
                        sqc = sq_p.tile([P, P], F32R, tag="sqc", name="sqc")
                        nc.scalar.activation(sqc[:], xa[:, d, tt * P:(tt + 1) * P], AF.Square)
                        nc.tensor.matmul(ps_mc[:], sqc[:], ones_c[:],
                                         start=(d == 0), stop=(d == ND - 1))
                    nc.scalar.activation(scol[:, tt:tt + 1], ps_mc[:, 0:1], AF.Sqrt,
                                         bias=eps_c[:], scale=1.0 / D)
                nc.vector.reciprocal(scol_i[:], scol[:])

                # --- K^T projection + RoPE -> KT_d (weights loaded once, 2 chunks) ---
                for h in range(H):
                    ps_k0 = psk.tile([P, CH], F32, tag="proj", name="ps_k0")
                    ps_k1 = psk.tile([P, CH], F32, tag="proj", name="ps_k1")
                    for d in range(ND):
                        wt = wstr.tile([P, P], F32R, tag="w", name="wt")
                        nc.sync.dma_start(wt[:], wk[d * P:(d + 1) * P, h * P:(h + 1) * P])
                        nc.tensor.matmul(ps_k0[:], wt[:], xa[:, d, 0:CH],
                                         start=(d == 0), stop=(d == ND - 1))
                        nc.tensor.matmul(ps_k1[:], wt[:], xa[:, d, CH:2 * CH],
                                         start=(d == 0), stop=(d == ND - 1))
                    for c, ps_k in ((0, ps_k0), (1, ps_k1)):
                        sl = slice(c * CH, (c + 1) * CH)
                        kt = ev_p.tile([P, CH], F32R, tag="kt", name="kt")
                        rope_evict(rope_p, ps_k, kt[:], cos1[:, sl], sin1[:, sl])
                        nc.sync.dma_start(KT_d[h, :, t0 + c * CH:t0 + (c + 1) * CH], kt[:])

                # --- V projection (token-major, s-scaled) -> V_d ---
                for g in range(4):
                    wvg = wvg_p.tile([P, ND, 4 * HD], F32R, tag="wvg", name="wvg")
                    nc.sync.dma_start(
                        wvg[:], wv.ap()[:, g * 4 * HD:(g + 1) * 4 * HD]
                        .rearrange("(a p) n -> p a n", p=P))
                    for tt in range(NT // 2):
                        ps_v = psv.tile([P, 4 * HD], F32, tag="vproj", name="ps_v")
                        for d in range(ND):
                            nc.tensor.matmul(ps_v[:], xa[:, d, tt * P:(tt + 1) * P],
                                             wvg[:, d, :],
                                             start=(d == 0), stop=(d == ND - 1))
                        vt = ev_p.tile([P, 4 * HD], F32R, tag="vt", name="vt")
                        nc.vector.tensor_scalar_mul(vt[:], ps_v[:], scol_i[:, tt:tt + 1])
                        nc.sync.dma_start(
                            V_d[t0 + tt * P:t0 + (tt + 1) * P, g * 4 * HD:(g + 1) * 4 * HD],
                            vt[:])

        # ================= Phase B: Q proj + attention =================
        attnp_cm = tc.tile_pool(name="attnp", bufs=1)
        attnp = attnp_cm.__enter__()
        attnT = attnp.tile([P, H, CH], F32R)

        with tc.tile_pool(name="qpool", bufs=1) as qpool, \
             tc.tile_pool(name="sqo", bufs=2) as sqo_p, \
             tc.tile_pool(name="wstrq", bufs=4) as wstrq, \
             tc.tile_pool(name="ropeq", bufs=2) as ropeq, \
             tc.tile_pool(name="kv", bufs=2) as kv_p, \
             tc.tile_pool(name="et", bufs=4) as et_p, \
             tc.tile_pool(name="bi", bufs=2) as bi_p, \
             tc.tile_pool(name="psq", bufs=2, space="PSUM") as psq, \
             tc.tile_pool(name="ps_s", bufs=2, space="PSUM") as ps_s, \
             tc.tile_pool(name="ps_o", bufs=2, space="PSUM") as ps_o, \
             tc.tile_pool(name="ps_d", bufs=1, space="PSUM") as ps_d, \
             tc.tile_pool(name="pssq", bufs=1, space="PSUM") as pssq:

            QTr = qpool.tile([P, H, CH], F32R)
            # own-chunk rms scale + rope tables
            srow_o = rms_row(sqo_p, pssq, xo[:], CH, ones_c, eps_c)
            sbo = sqo_p.tile([HF, CH], F32, tag="sb", name="sbo")
            nc.gpsimd.partition_broadcast(sbo[:], srow_o[:])
            cos1o = qpool.tile([HF, CH], F32)
            sin1o = qpool.tile([HF, CH], F32)
            cto = sqo_p.tile([HF, CH], F32, tag="ct", name="cto")
            nc.sync.dma_start(cto[:], cosT_own[:])
            sto = sqo_p.tile([HF, CH], F32, tag="st", name="sto")
            nc.sync.dma_start(sto[:], sinT_own[:])
            nc.vector.tensor_tensor(cos1o[:], cto[:], sbo[:], ALU.mult)
            nc.vector.tensor_tensor(sin1o[:], sto[:], sbo[:], ALU.mult)

            for h in range(H):
                ps_q = psq.tile([P, CH], F32, tag="qproj", name="ps_q")
                for d in range(ND):
                    wt = wstrq.tile([P, P], F32R, tag="w", name="wtq")
                    nc.sync.dma_start(wt[:], wq[d * P:(d + 1) * P, h * P:(h + 1) * P])
                    nc.tensor.matmul(ps_q[:], wt[:], xo[:, d, :],
                                     start=(d == 0), stop=(d == ND - 1))
                rope_evict(ropeq, ps_q, QTr[:, h, :], cos1o[:], sin1o[:])

            for h in range(H):
                ktr = kv_p.tile([P, S], F32R, tag="ktr", name="ktr")
                nc.sync.dma_start(ktr[:], KT_d[h, :, :])
                vh = kv_p.tile([P, NT, HD], F32R, tag="vh", name="vh")
                nc.sync.dma_start(
                    vh[:], V_d[:].rearrange("(a p) d -> p a d", p=P)[:, :, h * HD:(h + 1) * HD])
                po = ps_o.tile([P, CH], F32, tag="o", name="po")
                pd = ps_d.tile([1, CH], F32, tag="d", name="pd")
                for kt in range(NT):
                    ps_sc = ps_s.tile([P, CH], F32, tag="s", name="ps_sc")
                    nc.tensor.matmul(ps_sc[:], ktr[:, kt * P:(kt + 1) * P], QTr[:, h, :],
                                     start=True, stop=True)
                    e = et_p.tile([P, CH], F32R, tag="e", name="e")
                    nc.scalar.activation(e[:], ps_sc[:], AF.Exp, scale=ISQ)
                    nc.tensor.matmul(po[:], vh[:, kt, :], e[:],
                                     start=(kt == 0), stop=(kt == NT - 1))
                    nc.tensor.matmul(pd[:], ones_c[:, 0:1], e[:],
                                     start=(kt == 0), stop=(kt == NT - 1))
                inv = bi_p.tile([1, CH], F32, tag="inv", name="inv")
                nc.vector.reciprocal(inv[:], pd[:])
                binv = bi_p.tile([P, CH], F32, tag="binv", name="binv")
                nc.gpsimd.partition_broadcast(binv[:], inv[:])
                nc.vector.tensor_tensor(attnT[:, h, :], po[:], binv[:], ALU.mult)

        # ================= Phase C: wo + residual -> hT =================
        hp_cm = tc.tile_pool(name="hp", bufs=1, side="right")
        hp = hp_cm.__enter__()
        hT = hp.tile([P, ND, CH], F32)

        with tc.tile_pool(name="wstr2", bufs=4) as wstr2, \
             tc.tile_pool(name="ps_w", bufs=2, space="PSUM") as ps_w:
            for j in range(ND):
                ps_h = ps_w.tile([P, CH], F32, tag="wo", name="ps_h")
                for d in range(ND):
                    wt = wstr2.tile([P, P], F32R, tag="w", name="wt2")
                    nc.sync.dma_start(wt[:], wo[d * P:(d + 1) * P, j * P:(j + 1) * P])
                    nc.tensor.matmul(ps_h[:], wt[:], attnT[:, d, :],
                                     start=(d == 0), stop=(d == ND - 1))
                nc.vector.tensor_tensor(hT[:, j, :], ps_h[:], xo[:, j, :], ALU.add)

        attnp_cm.__exit__(None, None, None)
        own_cm.__exit__(None, None, None)

        # ================= Phase D: FFN =================
        with tc.tile_pool(name="fp", bufs=1) as fp, \
             tc.tile_pool(name="swp", bufs=1) as sw_p, \
             tc.tile_pool(name="fstr", bufs=4) as fstr, \
             tc.tile_pool(name="fev", bufs=2) as fev, \
             tc.tile_pool(name="fsc", bufs=2) as fsc, \
             tc.tile_pool(name="ps_u", bufs=2, space="PSUM") as ps_u, \
             tc.tile_pool(name="ps_g", bufs=2, space="PSUM") as ps_g, \
             tc.tile_pool(name="ps_y", bufs=2, space="PSUM") as ps_y, \
             tc.tile_pool(name="ps_n", bufs=1, space="PSUM") as ps_n:

            s2 = rms_row(fsc, ps_n, hT[:], CH, ones_c, eps_c)
            bs2 = fsc.tile([P, CH], F32, tag="bs2", name="bs2")
            nc.gpsimd.partition_broadcast(bs2[:], s2[:])
            hnT = fp.tile([P, ND, CH], F32R)
            for d in range(ND):
                nc.vector.tensor_tensor(hnT[:, d, :], hT[:, d, :], bs2[:], ALU.mult)

            swt = sw_p.tile([P, NHID, CH], F32R)
            for k in range(NHID):
                ps_uu = ps_u.tile([P, CH], F32, tag="u", name="ps_uu")
                for d in range(ND):
                    wt = fstr.tile([P, P], F32R, tag="w1", name="wt1")
                    nc.sync.dma_start(wt[:], w1[d * P:(d + 1) * P, k * P:(k + 1) * P])
                    nc.tensor.matmul(ps_uu[:], wt[:], hnT[:, d, :],
                                     start=(d == 0), stop=(d == ND - 1))
                su = fev.tile([P, CH], F32R, tag="su", name="su")
                nc.scalar.activation(su[:], ps_uu[:], AF.Silu)
                ps_gg = ps_g.tile([P, CH], F32, tag="g", name="ps_gg")
                for d in range(ND):
                    wt = fstr.tile([P, P], F32R, tag="w3", name="wt3")
                    nc.sync.dma_start(wt[:], w3[d * P:(d + 1) * P, k * P:(k + 1) * P])
                    nc.tensor.matmul(ps_gg[:], wt[:], hnT[:, d, :],
                                     start=(d == 0), stop=(d == ND - 1))
                nc.vector.tensor_tensor(swt[:, k, :], ps_gg[:], su[:], ALU.mult)

            for j in range(ND):
                ps_yy = ps_y.tile([P, CH], F32, tag="y", name="ps_yy")
                for k in range(NHID):
                    wt = fstr.tile([P, P], F32R, tag="w2", name="wt2f")
                    nc.sync.dma_start(wt[:], w2[k * P:(k + 1) * P, j * P:(j + 1) * P])
                    nc.tensor.matmul(ps_yy[:], wt[:], swt[:, k, :],
                                     start=(k == 0), stop=(k == NHID - 1))
                yt = fev.tile([P, CH], F32, tag="yt", name="yt")
                nc.vector.tensor_tensor(yt[:], ps_yy[:], hT[:, j, :], ALU.add)
                nc.sync.dma_start(yT[j * P:(j + 1) * P, :], yt[:])

        hp_cm.__exit__(None, None, None)
        consts_cm.__exit__(None, None, None)
        dram_cm.__exit__(None, None, None)

    nc.compile()
    return nc


class _Runner:
    def __init__(self, nc, n_cores=NCORES):
        import jax
        from jax.sharding import Mesh, PartitionSpec
        from jax.experimental.shard_map import shard_map
        from concourse import mybir
        from concourse.bass2jax import _bass_exec_p, install_neuronx_cc_hook, partition_id_tensor

        install_neuronx_cc_hook()
        self.nc = nc
        self.n_cores = n_cores
        partition_name = nc.partition_id_tensor.name if nc.partition_id_tensor else None
        in_names, out_names, out_avals = [], [], []
        for alloc in nc.m.functions[0].allocations:
            if not isinstance(alloc, mybir.MemoryLocationSet):
                continue
            name = alloc.memorylocations[0].name
            if alloc.kind == "ExternalInput":
                if name != partition_name and name != (nc.dbg_addr.name if nc.dbg_addr else None):
                    in_names.append(name)
            elif alloc.kind == "ExternalOutput":
                out_names.append(name)
                out_avals.append(jax.core.ShapedArray(tuple(alloc.tensor_shape), mybir.dt.np(alloc.dtype)))
        self.in_names, self.out_names, self.out_avals = in_names, out_names, out_avals
        has_dbg = nc.dbg_addr is not None
        all_in = tuple(in_names + out_names
                       + ([nc.dbg_addr.name] if has_dbg else [])
                       + ([partition_name] if partition_name else []))

        def _body(*args):
            import jax.numpy as jnp
            operands = list(args)
            if has_dbg:
                operands.append(jnp.zeros((1, 2), jnp.uint32))
            if partition_name is not None:
                operands.append(partition_id_tensor())
            outs = _bass_exec_p.bind(
                *operands,
                out_avals=tuple(out_avals),
                in_names=all_in,
                out_names=tuple(out_names),
                lowering_input_output_aliases=(),
                sim_require_finite=False,
                sim_require_nnan=False,
                nc=nc,
            )
            return tuple(outs)

        devices = jax.devices()[:n_cores]
        self.mesh = Mesh(np.asarray(devices), ("core",))
        n_params = len(in_names)
        in_specs = (PartitionSpec("core"),) * (n_params + len(out_names))
        out_specs = (PartitionSpec("core"),) * len(out_names)
        self.fn = jax.jit(
            shard_map(_body, mesh=self.mesh, in_specs=in_specs, out_specs=out_specs,
                      check_rep=False),
            keep_unused=True,
        )

    def stage(self, in_maps):
        import jax
        from jax.sharding import PartitionSpec
        n = self.n_cores
        concat_in = [
            np.concatenate([np.asarray(in_maps[c][name]) for c in range(n)], axis=0)
            for name in self.in_names
        ]
        concat_zeros = [np.zeros((n * a.shape[0], *a.shape[1:]), a.dtype) for a in self.out_avals]
        sharding = jax.sharding.NamedSharding(self.mesh, PartitionSpec("core"))
        staged = [jax.device_put(x, sharding) for x in concat_in + concat_zeros]
        for x in staged:
            x.block_until_ready()
        return staged

    def run_staged(self, staged):
        import jax
        out = self.fn(*staged)
        jax.block_until_ready(out)
        return out

    def run(self, in_maps):
        out_arrs = self.run_staged(self.stage(in_maps))
        n = self.n_cores
        return [
            {name: np.asarray(out_arrs[i]).reshape(n, *self.out_avals[i].shape)[c]
             for i, name in enumerate(self.out_names)}
            for c in range(n)
        ]


def _perm_pairs():
    p = np.arange(D).reshape(H, HD // 2, 2)
    return np.concatenate([p[..., 0], p[..., 1]], axis=-1).reshape(-1)


def make_in_maps(x, freqs_cos, freqs_sin, wq, wk, wv, wo, w1, w2, w3,
                 attn_norm_w, ffn_norm_w):
    f32 = np.float32
    x = np.asarray(x, f32)
    cos = np.ascontiguousarray(np.asarray(freqs_cos, f32).reshape(S, HD // 2).T)
    sin = np.ascontiguousarray(np.asarray(freqs_sin, f32).reshape(S, HD // 2).T)
    perm = _perm_pairs()
    anw = np.asarray(attn_norm_w, f32)
    fnw = np.asarray(ffn_norm_w, f32)
    wq_p = np.ascontiguousarray((np.asarray(wq, f32) * anw[:, None])[:, perm])
    wk_p = np.ascontiguousarray((np.asarray(wk, f32) * anw[:, None])[:, perm])
    wv_f = np.ascontiguousarray(np.asarray(wv, f32) * anw[:, None])
    wo_f = np.ascontiguousarray(np.asarray(wo, f32))
    w1_f = np.ascontiguousarray(np.asarray(w1, f32) * fnw[:, None])
    w3_f = np.ascontiguousarray(np.asarray(w3, f32) * fnw[:, None])
    w2_f = np.ascontiguousarray(np.asarray(w2, f32))
    xT = [np.ascontiguousarray(x[b].T) for b in range(B)]
    in_maps = []
    for core in range(NCORES):
        b, c = divmod(core, 4)
        in_maps.append({
            "xT": xT[b],
            "xT_own": np.ascontiguousarray(xT[b][:, c * CH:(c + 1) * CH]),
            "cosT": cos, "sinT": sin,
            "cosT_own": np.ascontiguousarray(cos[:, c * CH:(c + 1) * CH]),
            "sinT_own": np.ascontiguousarray(sin[:, c * CH:(c + 1) * CH]),
            "wq": wq_p, "wk": wk_p, "wv": wv_f, "wo": wo_f,
            "w1": w1_f, "w3": w3_f, "w2": w2_f,
        })
    return in_maps


def get_runner():
    if "runner" not in _cache:
        _patch_ldw_opt()
        nc = build_nc()
        _cache["runner"] = _Runner(nc, NCORES)
    return _cache["runner"]


def kernel(**inputs) -> np.ndarray:
    r = get_runner()
    in_maps = make_in_maps(**inputs)
    res = r.run(in_maps)
    y = np.empty((B, S, D), np.float32)
    for core in range(NCORES):
        b, c = divmod(core, 4)
        y[b, c * CH:(c + 1) * CH, :] = res[core]["yT"].T
    return y


# revision 6
# speedup vs baseline: 1.3213x; 1.3213x over previous
"""Bass/Tile Trainium2 kernel for nn_DiffusionTransformerBlock.

kernel(**inputs) takes the FULL unsharded inputs of the reference model
and returns the FULL [B,S,D] output, distributing work across 8
NeuronCores internally (SPMD, no collectives).

Sharding: core = b*4 + c handles batch b, query-token chunk c (512
tokens), recomputing K/V for its batch's full sequence (communication-
free; +33% matmul flops but no collective latency/risk).

Dataflow is fully transposed ([feature, token] layout) so projections
need no on-chip transposes; RoPE runs on deinterleaved head dims via a
host-side column permutation of wq/wk; the rmsnorm scale is folded into
the cos/sin tables (Q,K), a per-partition scale at V eviction, and the
norm weights are folded into projection weights host-side. Matmuls run
in float32r (full PE rate, ~13-bit mantissa).
"""
import sys

sys.path.insert(0, "/opt/trn_rl_repo")

import numpy as np

B, S, D, H = 2, 2048, 2048, 16
HD = D // H          # 128
HID = 5632
EPS = 1e-5
NCORES = 8
CH = S // 4          # 512 own tokens per core
P = 128
NT = S // P          # 16 token tiles
ND = D // P          # 16 feature tiles
NHID = HID // P      # 44
HF = HD // 2         # 64
ISQ = float(1.0 / np.sqrt(HD))

_cache = {}


def _patch_ldw_opt():
    import concourse.bass_utils as bass_utils
    if getattr(bass_utils, "_ldw_opt_patched", False):
        return
    orig = bass_utils.run_command

    def patched(argv, **kwargs):
        argv = [a.replace("--enable-ldw-opt=false", "--enable-ldw-opt=true") for a in argv]
        return orig(argv, **kwargs)

    bass_utils.run_command = patched
    bass_utils._ldw_opt_patched = True


def build_nc():
    import concourse.tile as tile
    from concourse import bacc, mybir

    dt = mybir.dt
    F32, F32R = dt.float32, dt.float32r
    AF = mybir.ActivationFunctionType
    ALU = mybir.AluOpType

    nc = bacc.Bacc("TRN2", target_bir_lowering=False, debug=False, num_devices=NCORES)

    xT = nc.dram_tensor("xT", [D, S], F32R, kind="ExternalInput")
    xT_own = nc.dram_tensor("xT_own", [D, CH], F32R, kind="ExternalInput")
    cosT = nc.dram_tensor("cosT", [HF, S], F32, kind="ExternalInput")
    sinT = nc.dram_tensor("sinT", [HF, S], F32, kind="ExternalInput")
    cosT_own = nc.dram_tensor("cosT_own", [HF, CH], F32, kind="ExternalInput")
    sinT_own = nc.dram_tensor("sinT_own", [HF, CH], F32, kind="ExternalInput")
    # weights pre-tiled host-side: [outblk, 128 part(din%128), din//128, outcols]
    wq = nc.dram_tensor("wq", [H, P, ND, HD], F32R, kind="ExternalInput")
    wk = nc.dram_tensor("wk", [H, P, ND, HD], F32R, kind="ExternalInput")
    wv = nc.dram_tensor("wv", [4, P, ND, 4 * HD], F32R, kind="ExternalInput")
    wo = nc.dram_tensor("wo", [ND, P, ND, P], F32R, kind="ExternalInput")
    w1 = nc.dram_tensor("w1", [NHID, P, ND, P], F32R, kind="ExternalInput")
    w3 = nc.dram_tensor("w3", [NHID, P, ND, P], F32R, kind="ExternalInput")
    w2 = nc.dram_tensor("w2", [ND, P, NHID, P], F32R, kind="ExternalInput")
    yT = nc.dram_tensor("yT", [D, CH], F32, kind="ExternalOutput")

    def rope_evict(rope_p, ps_in, out_ap, cos1, sin1):
        t1 = rope_p.tile([HF, cos1.shape[-1]], F32, tag="t1", name="t1")
        t2 = rope_p.tile([HF, cos1.shape[-1]], F32, tag="t2", name="t2")
        t3 = rope_p.tile([HF, cos1.shape[-1]], F32, tag="t1", name="t3")
        t4 = rope_p.tile([HF, cos1.shape[-1]], F32, tag="t2", name="t4")
        a = ps_in[0:HF, :]
        b = ps_in[HF:HD, :]
        nc.vector.tensor_tensor(t1[:], a, cos1, ALU.mult)
        nc.vector.tensor_tensor(t2[:], b, sin1, ALU.mult)
        nc.vector.tensor_tensor(t3[:], a, sin1, ALU.mult)
        nc.vector.tensor_tensor(t4[:], b, cos1, ALU.mult)
        nc.vector.tensor_tensor(out_ap[0:HF, :], t1[:], t2[:], ALU.subtract)
        nc.vector.tensor_tensor(out_ap[HF:HD, :], t3[:], t4[:], ALU.add)

    def rms_row(pool, pspool, src3d, n, ones_c, eps_c):
        """1/sqrt(mean_d(src^2)+eps) along partitions+tiles -> [1, n] f32 tile."""
        ps_ms = pspool.tile([1, n], F32, tag="msrow", name="ps_ms")
        for d in range(ND):
            sq = pool.tile([P, n], F32R, tag="sq", name="sq")
            nc.scalar.activation(sq[:], src3d[:, d, :], AF.Square)
            nc.tensor.matmul(ps_ms[:], ones_c[:, 0:1], sq[:],
                             start=(d == 0), stop=(d == ND - 1))
        rms = pool.tile([1, n], F32, tag="rms", name="rms")
        nc.scalar.activation(rms[:], ps_ms[:], AF.Sqrt, bias=eps_c[0:1, :], scale=1.0 / D)
        srow = pool.tile([1, n], F32, tag="srow", name="srow")
        nc.vector.reciprocal(srow[:], rms[:])
        return srow

    with tile.TileContext(nc) as tc:
        dram_cm = tc.tile_pool(name="dram", bufs=1, space="DRAM")
        dram = dram_cm.__enter__()
        consts_cm = tc.tile_pool(name="consts", bufs=1)
        consts = consts_cm.__enter__()
        own_cm = tc.tile_pool(name="own", bufs=1)
        own = own_cm.__enter__()

        KT_d = dram.tile([H, HD, S], F32R)
        V_d = dram.tile([S, D], F32R)

        ones_f = consts.tile([P, 2], F32)
        nc.vector.memset(ones_f[:], 1.0)
        ones_c = consts.tile([P, 2], F32R)
        nc.vector.tensor_copy(ones_c[:], ones_f[:])
        eps_c = consts.tile([P, 1], F32)
        nc.vector.memset(eps_c[:], EPS)

        xo = own.tile([P, ND, CH], F32R)
        nc.scalar.dma_start(xo[:], xT_own.ap().rearrange("(a p) t -> p a t", p=P))

        # ================= Phase A: full-seq s, K^T -> HBM, V -> HBM =================
        with tc.tile_pool(name="xa", bufs=1) as xa_p, \
             tc.tile_pool(name="sqp", bufs=2) as sq_p, \
             tc.tile_pool(name="wstr", bufs=2) as wstr, \
             tc.tile_pool(name="wvg", bufs=1) as wvg_p, \
             tc.tile_pool(name="rope", bufs=2) as rope_p, \
             tc.tile_pool(name="ev", bufs=3) as ev_p, \
             tc.tile_pool(name="sc", bufs=1) as sc_p, \
             tc.tile_pool(name="psk", bufs=4, space="PSUM") as psk, \
             tc.tile_pool(name="psv", bufs=2, space="PSUM") as psv, \
             tc.tile_pool(name="pss", bufs=1, space="PSUM") as pss:

            for half in range(2):
                t0 = half * (S // 2)
                HS = S // 2
                xa = xa_p.tile([P, ND, HS], F32R, tag="xa", name="xa")
                nc.scalar.dma_start(
                    xa[:], xT.ap()[:, t0:t0 + HS].rearrange("(a p) t -> p a t", p=P))

                # --- per-512-chunk s_row -> cos1/sin1 (norm scale folded) ---
                cos1 = sc_p.tile([HF, HS], F32, tag="cos1", name="cos1")
                sin1 = sc_p.tile([HF, HS], F32, tag="sin1", name="sin1")
                for c in range(HS // CH):
                    sl = slice(c * CH, (c + 1) * CH)
                    srow = rms_row(sq_p, pss, xa[:, :, sl], CH, ones_c, eps_c)
                    sb = sc_p.tile([HF, CH], F32, tag="sb", name="sb")
                    nc.gpsimd.partition_broadcast(sb[:], srow[:])
                    ct = sq_p.tile([HF, CH], F32, tag="ct", name="ct")
                    nc.scalar.dma_start(ct[:], cosT[:, t0 + c * CH:t0 + (c + 1) * CH])
                    st = sq_p.tile([HF, CH], F32, tag="st", name="st")
                    nc.scalar.dma_start(st[:], sinT[:, t0 + c * CH:t0 + (c + 1) * CH])
                    nc.vector.tensor_tensor(cos1[:, sl], ct[:], sb[:], ALU.mult)
                    nc.vector.tensor_tensor(sin1[:, sl], st[:], sb[:], ALU.mult)

                # --- s_col (per-token 1/rms as column) for V scaling ---
                scol_i = sc_p.tile([P, NT // 2], F32, tag="scoli", name="scol_i")
                scol = sc_p.tile([P, NT // 2], F32, tag="scol", name="scol")
                for tt in range(NT // 2):
                    ps_mc = pss.tile([P, 2], F32, tag="mscol", name="ps_mc")
                    for d in range(ND):
                        sqc = sq_p.tile([P, P], F32R, tag="sqc", name="sqc")
                        nc.scalar.activation(sqc[:], xa[:, d, tt * P:(tt + 1) * P], AF.Square)
                        nc.tensor.matmul(ps_mc[:], sqc[:], ones_c[:],
                                         start=(d == 0), stop=(d == ND - 1))
                    nc.scalar.activation(scol[:, tt:tt + 1], ps_mc[:, 0:1], AF.Sqrt,
                                         bias=eps_c[:], scale=1.0 / D)
                nc.vector.reciprocal(scol_i[:], scol[:])

                # --- K^T projection + RoPE -> KT_d (weights loaded once, 2 chunks) ---
                for h in range(H):
                    wkh = wstr.tile([P, ND, HD], F32R, tag="w", name="wkh")
                    nc.sync.dma_start(wkh[:], wk.ap()[h])
                    ps_k0 = psk.tile([P, CH], F32, tag="proj", name="ps_k0")
                    ps_k1 = psk.tile([P, CH], F32, tag="proj", name="ps_k1")
                    for d in range(ND):
                        nc.tensor.matmul(ps_k0[:], wkh[:, d, :], xa[:, d, 0:CH],
                                         start=(d == 0), stop=(d == ND - 1))
                        nc.tensor.matmul(ps_k1[:], wkh[:, d, :], xa[:, d, CH:2 * CH],
                                         start=(d == 0), stop=(d == ND - 1))
                    for c, ps_k in ((0, ps_k0), (1, ps_k1)):
                        sl = slice(c * CH, (c + 1) * CH)
                        kt = ev_p.tile([P, CH], F32R, tag="kt", name="kt")
                        rope_evict(rope_p, ps_k, kt[:], cos1[:, sl], sin1[:, sl])
                        nc.gpsimd.dma_start(KT_d[h, :, t0 + c * CH:t0 + (c + 1) * CH], kt[:])

                # --- V projection (token-major, s-scaled) -> V_d ---
                for g in range(4):
                    wvg = wvg_p.tile([P, ND, 4 * HD], F32R, tag="wvg", name="wvg")
                    nc.sync.dma_start(wvg[:], wv.ap()[g])
                    for tt in range(NT // 2):
                        ps_v = psv.tile([P, 4 * HD], F32, tag="vproj", name="ps_v")
                        for d in range(ND):
                            nc.tensor.matmul(ps_v[:], xa[:, d, tt * P:(tt + 1) * P],
                                             wvg[:, d, :],
                                             start=(d == 0), stop=(d == ND - 1))
                        vt = ev_p.tile([P, 4 * HD], F32R, tag="vt", name="vt")
                        nc.vector.tensor_scalar_mul(vt[:], ps_v[:], scol_i[:, tt:tt + 1])
                        nc.gpsimd.dma_start(
                            V_d[t0 + tt * P:t0 + (tt + 1) * P, g * 4 * HD:(g + 1) * 4 * HD],
                            vt[:])

        # ================= Phase B: Q proj + attention =================
        attnp_cm = tc.tile_pool(name="attnp", bufs=1)
        attnp = attnp_cm.__enter__()
        attnT = attnp.tile([P, H, CH], F32R)

        with tc.tile_pool(name="qpool", bufs=1) as qpool, \
             tc.tile_pool(name="sqo", bufs=2) as sqo_p, \
             tc.tile_pool(name="wstrq", bufs=2) as wstrq, \
             tc.tile_pool(name="ropeq", bufs=2) as ropeq, \
             tc.tile_pool(name="kv", bufs=2) as kv_p, \
             tc.tile_pool(name="et", bufs=4) as et_p, \
             tc.tile_pool(name="bi", bufs=2) as bi_p, \
             tc.tile_pool(name="psq", bufs=2, space="PSUM") as psq, \
             tc.tile_pool(name="ps_s", bufs=2, space="PSUM") as ps_s, \
             tc.tile_pool(name="ps_o", bufs=2, space="PSUM") as ps_o, \
             tc.tile_pool(name="ps_d", bufs=1, space="PSUM") as ps_d, \
             tc.tile_pool(name="pssq", bufs=1, space="PSUM") as pssq:

            QTr = qpool.tile([P, H, CH], F32R)
            # own-chunk rms scale + rope tables
            srow_o = rms_row(sqo_p, pssq, xo[:], CH, ones_c, eps_c)
            sbo = sqo_p.tile([HF, CH], F32, tag="sb", name="sbo")
            nc.gpsimd.partition_broadcast(sbo[:], srow_o[:])
            cos1o = qpool.tile([HF, CH], F32)
            sin1o = qpool.tile([HF, CH], F32)
            cto = sqo_p.tile([HF, CH], F32, tag="ct", name="cto")
            nc.scalar.dma_start(cto[:], cosT_own[:])
            sto = sqo_p.tile([HF, CH], F32, tag="st", name="sto")
            nc.scalar.dma_start(sto[:], sinT_own[:])
            nc.vector.tensor_tensor(cos1o[:], cto[:], sbo[:], ALU.mult)
            nc.vector.tensor_tensor(sin1o[:], sto[:], sbo[:], ALU.mult)

            for h in range(H):
                wqh = wstrq.tile([P, ND, HD], F32R, tag="w", name="wqh")
                nc.sync.dma_start(wqh[:], wq.ap()[h])
                ps_q = psq.tile([P, CH], F32, tag="qproj", name="ps_q")
                for d in range(ND):
                    nc.tensor.matmul(ps_q[:], wqh[:, d, :], xo[:, d, :],
                                     start=(d == 0), stop=(d == ND - 1))
                rope_evict(ropeq, ps_q, QTr[:, h, :], cos1o[:], sin1o[:])

            for h in range(H):
                ktr = kv_p.tile([P, S], F32R, tag="ktr", name="ktr")
                nc.scalar.dma_start(ktr[:], KT_d[h, :, :])
                vh = kv_p.tile([P, NT, HD], F32R, tag="vh", name="vh")
                nc.scalar.dma_start(
                    vh[:], V_d[:].rearrange("(a p) d -> p a d", p=P)[:, :, h * HD:(h + 1) * HD])
                po = ps_o.tile([P, CH], F32, tag="o", name="po")
                pd = ps_d.tile([1, CH], F32, tag="d", name="pd")
                for kt in range(NT):
                    ps_sc = ps_s.tile([P, CH], F32, tag="s", name="ps_sc")
                    nc.tensor.matmul(ps_sc[:], ktr[:, kt * P:(kt + 1) * P], QTr[:, h, :],
                                     start=True, stop=True)
                    e = et_p.tile([P, CH], F32R, tag="e", name="e")
                    nc.scalar.activation(e[:], ps_sc[:], AF.Exp, scale=ISQ)
                    nc.tensor.matmul(po[:], vh[:, kt, :], e[:],
                                     start=(kt == 0), stop=(kt == NT - 1))
                    nc.tensor.matmul(pd[:], ones_c[:, 0:1], e[:],
                                     start=(kt == 0), stop=(kt == NT - 1))
                inv = bi_p.tile([1, CH], F32, tag="inv", name="inv")
                nc.vector.reciprocal(inv[:], pd[:])
                binv = bi_p.tile([P, CH], F32, tag="binv", name="binv")
                nc.gpsimd.partition_broadcast(binv[:], inv[:])
                nc.vector.tensor_tensor(attnT[:, h, :], po[:], binv[:], ALU.mult)

        # ================= Phase C: wo + residual -> hT =================
        hp_cm = tc.tile_pool(name="hp", bufs=1, side="right")
        hp = hp_cm.__enter__()
        hT = hp.tile([P, ND, CH], F32)

        with tc.tile_pool(name="wstr2", bufs=2) as wstr2, \
             tc.tile_pool(name="ps_w", bufs=2, space="PSUM") as ps_w:
            for j in range(ND):
                woj = wstr2.tile([P, ND, P], F32R, tag="w", name="woj")
                nc.sync.dma_start(woj[:], wo.ap()[j])
                ps_h = ps_w.tile([P, CH], F32, tag="wo", name="ps_h")
                for d in range(ND):
                    nc.tensor.matmul(ps_h[:], woj[:, d, :], attnT[:, d, :],
                                     start=(d == 0), stop=(d == ND - 1))
                nc.vector.tensor_tensor(hT[:, j, :], ps_h[:], xo[:, j, :], ALU.add)

        attnp_cm.__exit__(None, None, None)
        own_cm.__exit__(None, None, None)

        # ================= Phase D: FFN =================
        with tc.tile_pool(name="swp", bufs=1) as sw_p:
            swt = sw_p.tile([P, NHID, CH], F32R)
            with tc.tile_pool(name="fp", bufs=1) as fp, \
                 tc.tile_pool(name="fstr13", bufs=2) as fstr13, \
                 tc.tile_pool(name="fev1", bufs=2) as fev1, \
                 tc.tile_pool(name="fsc", bufs=2) as fsc, \
                 tc.tile_pool(name="ps_u", bufs=2, space="PSUM") as ps_u, \
                 tc.tile_pool(name="ps_g", bufs=2, space="PSUM") as ps_g, \
                 tc.tile_pool(name="ps_n", bufs=1, space="PSUM") as ps_n:

                s2 = rms_row(fsc, ps_n, hT[:], CH, ones_c, eps_c)
                bs2 = fsc.tile([P, CH], F32, tag="bs2", name="bs2")
                nc.gpsimd.partition_broadcast(bs2[:], s2[:])
                hnT = fp.tile([P, ND, CH], F32R)
                for d in range(ND):
                    nc.vector.tensor_tensor(hnT[:, d, :], hT[:, d, :], bs2[:], ALU.mult)

                for k in range(NHID):
                    w1k = fstr13.tile([P, ND, P], F32R, tag="w1", name="w1k")
                    nc.sync.dma_start(w1k[:], w1.ap()[k])
                    ps_uu = ps_u.tile([P, CH], F32, tag="u", name="ps_uu")
                    for d in range(ND):
                        nc.tensor.matmul(ps_uu[:], w1k[:, d, :], hnT[:, d, :],
                                         start=(d == 0), stop=(d == ND - 1))
                    su = fev1.tile([P, CH], F32R, tag="su", name="su")
                    nc.scalar.activation(su[:], ps_uu[:], AF.Silu)
                    w3k = fstr13.tile([P, ND, P], F32R, tag="w3", name="w3k")
                    nc.sync.dma_start(w3k[:], w3.ap()[k])
                    ps_gg = ps_g.tile([P, CH], F32, tag="g", name="ps_gg")
                    for d in range(ND):
                        nc.tensor.matmul(ps_gg[:], w3k[:, d, :], hnT[:, d, :],
                                         start=(d == 0), stop=(d == ND - 1))
                    nc.vector.tensor_tensor(swt[:, k, :], ps_gg[:], su[:], ALU.mult)

            with tc.tile_pool(name="fstr2", bufs=2) as fstr2, \
                 tc.tile_pool(name="fev2", bufs=2) as fev2, \
                 tc.tile_pool(name="ps_y", bufs=2, space="PSUM") as ps_y:
                for j in range(ND):
                    w2j = fstr2.tile([P, NHID, P], F32R, tag="w2", name="w2j")
                    nc.sync.dma_start(w2j[:], w2.ap()[j])
                    ps_yy = ps_y.tile([P, CH], F32, tag="y", name="ps_yy")
                    for k in range(NHID):
                        nc.tensor.matmul(ps_yy[:], w2j[:, k, :], swt[:, k, :],
                                         start=(k == 0), stop=(k == NHID - 1))
                    yt = fev2.tile([P, CH], F32, tag="yt", name="yt")
                    nc.vector.tensor_tensor(yt[:], ps_yy[:], hT[:, j, :], ALU.add)
                    nc.sync.dma_start(yT[j * P:(j + 1) * P, :], yt[:])

        hp_cm.__exit__(None, None, None)
        consts_cm.__exit__(None, None, None)
        dram_cm.__exit__(None, None, None)

    nc.compile()
    return nc


class _Runner:
    def __init__(self, nc, n_cores=NCORES):
        import jax
        from jax.sharding import Mesh, PartitionSpec
        from jax.experimental.shard_map import shard_map
        from concourse import mybir
        from concourse.bass2jax import _bass_exec_p, install_neuronx_cc_hook, partition_id_tensor

        install_neuronx_cc_hook()
        self.nc = nc
        self.n_cores = n_cores
        partition_name = nc.partition_id_tensor.name if nc.partition_id_tensor else None
        in_names, out_names, out_avals = [], [], []
        for alloc in nc.m.functions[0].allocations:
            if not isinstance(alloc, mybir.MemoryLocationSet):
                continue
            name = alloc.memorylocations[0].name
            if alloc.kind == "ExternalInput":
                if name != partition_name and name != (nc.dbg_addr.name if nc.dbg_addr else None):
                    in_names.append(name)
            elif alloc.kind == "ExternalOutput":
                out_names.append(name)
                out_avals.append(jax.core.ShapedArray(tuple(alloc.tensor_shape), mybir.dt.np(alloc.dtype)))
        self.in_names, self.out_names, self.out_avals = in_names, out_names, out_avals
        has_dbg = nc.dbg_addr is not None
        all_in = tuple(in_names + out_names
                       + ([nc.dbg_addr.name] if has_dbg else [])
                       + ([partition_name] if partition_name else []))

        def _body(*args):
            import jax.numpy as jnp
            operands = list(args)
            if has_dbg:
                operands.append(jnp.zeros((1, 2), jnp.uint32))
            if partition_name is not None:
                operands.append(partition_id_tensor())
            outs = _bass_exec_p.bind(
                *operands,
                out_avals=tuple(out_avals),
                in_names=all_in,
                out_names=tuple(out_names),
                lowering_input_output_aliases=(),
                sim_require_finite=False,
                sim_require_nnan=False,
                nc=nc,
            )
            return tuple(outs)

        devices = jax.devices()[:n_cores]
        self.mesh = Mesh(np.asarray(devices), ("core",))
        n_params = len(in_names)
        in_specs = (PartitionSpec("core"),) * (n_params + len(out_names))
        out_specs = (PartitionSpec("core"),) * len(out_names)
        self.fn = jax.jit(
            shard_map(_body, mesh=self.mesh, in_specs=in_specs, out_specs=out_specs,
                      check_rep=False),
            keep_unused=True,
        )

    def stage(self, in_maps):
        import jax
        from jax.sharding import PartitionSpec
        n = self.n_cores
        concat_in = [
            np.concatenate([np.asarray(in_maps[c][name]) for c in range(n)], axis=0)
            for name in self.in_names
        ]
        concat_zeros = [np.zeros((n * a.shape[0], *a.shape[1:]), a.dtype) for a in self.out_avals]
        sharding = jax.sharding.NamedSharding(self.mesh, PartitionSpec("core"))
        staged = [jax.device_put(x, sharding) for x in concat_in + concat_zeros]
        for x in staged:
            x.block_until_ready()
        return staged

    def run_staged(self, staged):
        import jax
        out = self.fn(*staged)
        jax.block_until_ready(out)
        return out

    def run(self, in_maps):
        out_arrs = self.run_staged(self.stage(in_maps))
        n = self.n_cores
        return [
            {name: np.asarray(out_arrs[i]).reshape(n, *self.out_avals[i].shape)[c]
             for i, name in enumerate(self.out_names)}
            for c in range(n)
        ]


def _perm_pairs():
    p = np.arange(D).reshape(H, HD // 2, 2)
    return np.concatenate([p[..., 0], p[..., 1]], axis=-1).reshape(-1)


def make_in_maps(x, freqs_cos, freqs_sin, wq, wk, wv, wo, w1, w2, w3,
                 attn_norm_w, ffn_norm_w):
    f32 = np.float32
    x = np.asarray(x, f32)
    cos = np.ascontiguousarray(np.asarray(freqs_cos, f32).reshape(S, HD // 2).T)
    sin = np.ascontiguousarray(np.asarray(freqs_sin, f32).reshape(S, HD // 2).T)
    perm = _perm_pairs()
    anw = np.asarray(attn_norm_w, f32)
    fnw = np.asarray(ffn_norm_w, f32)
    def blk(w, nblk, ncols):
        # [din, dout] -> [nblk, 128, din//128, ncols]
        din = w.shape[0]
        return np.ascontiguousarray(
            w.reshape(din // P, P, nblk, ncols).transpose(2, 1, 0, 3))

    wq_p = blk((np.asarray(wq, f32) * anw[:, None])[:, perm], H, HD)
    wk_p = blk((np.asarray(wk, f32) * anw[:, None])[:, perm], H, HD)
    wv_f = blk(np.asarray(wv, f32) * anw[:, None], 4, 4 * HD)
    wo_f = blk(np.asarray(wo, f32), ND, P)
    w1_f = blk(np.asarray(w1, f32) * fnw[:, None], NHID, P)
    w3_f = blk(np.asarray(w3, f32) * fnw[:, None], NHID, P)
    w2_f = blk(np.asarray(w2, f32), ND, P)
    xT = [np.ascontiguousarray(x[b].T) for b in range(B)]
    in_maps = []
    for core in range(NCORES):
        b, c = divmod(core, 4)
        in_maps.append({
            "xT": xT[b],
            "xT_own": np.ascontiguousarray(xT[b][:, c * CH:(c + 1) * CH]),
            "cosT": cos, "sinT": sin,
            "cosT_own": np.ascontiguousarray(cos[:, c * CH:(c + 1) * CH]),
            "sinT_own": np.ascontiguousarray(sin[:, c * CH:(c + 1) * CH]),
            "wq": wq_p, "wk": wk_p, "wv": wv_f, "wo": wo_f,
            "w1": w1_f, "w3": w3_f, "w2": w2_f,
        })
    return in_maps


def get_runner():
    if "runner" not in _cache:
        _patch_ldw_opt()
        nc = build_nc()
        _cache["runner"] = _Runner(nc, NCORES)
    return _cache["runner"]


def kernel(**inputs) -> np.ndarray:
    r = get_runner()
    in_maps = make_in_maps(**inputs)
    res = r.run(in_maps)
    y = np.empty((B, S, D), np.float32)
    for core in range(NCORES):
        b, c = divmod(core, 4)
        y[b, c * CH:(c + 1) * CH, :] = res[core]["yT"].T
    return y


# revision 7
# speedup vs baseline: 52.7692x; 39.9375x over previous
"""Bass/Tile Trainium2 kernel for nn_DiffusionTransformerBlock.

kernel(**inputs) takes the FULL unsharded inputs of the reference model
and returns the FULL [B,S,D] output, distributing work across 8
NeuronCores internally (SPMD, no collectives).

Sharding: core = b*4 + c handles batch b, query-token chunk c (512
tokens), recomputing K/V for its batch's full sequence (communication-
free; +33% matmul flops but no collective latency/risk).

Dataflow is fully transposed ([feature, token] layout) so projections
need no on-chip transposes; RoPE runs on deinterleaved head dims via a
host-side column permutation of wq/wk; the rmsnorm scale is folded into
the cos/sin tables (Q,K), a per-partition scale at V eviction, and the
norm weights are folded into projection weights host-side. Matmuls run
in float32r (full PE rate, ~13-bit mantissa).
"""
import sys

sys.path.insert(0, "/opt/trn_rl_repo")

import numpy as np

B, S, D, H = 2, 2048, 2048, 16
HD = D // H          # 128
HID = 5632
EPS = 1e-5
NCORES = 8
CH = S // 4          # 512 own tokens per core
P = 128
NT = S // P          # 16 token tiles
ND = D // P          # 16 feature tiles
NHID = HID // P      # 44
HF = HD // 2         # 64
ISQ = float(1.0 / np.sqrt(HD))

_cache = {}


def _patch_ldw_opt():
    import concourse.bass_utils as bass_utils
    if getattr(bass_utils, "_ldw_opt_patched", False):
        return
    orig = bass_utils.run_command

    def patched(argv, **kwargs):
        argv = [a.replace("--enable-ldw-opt=false", "--enable-ldw-opt=true") for a in argv]
        return orig(argv, **kwargs)

    bass_utils.run_command = patched
    bass_utils._ldw_opt_patched = True


def build_nc(loop_iters=None):
    import concourse.tile as tile
    from concourse import bacc, mybir

    dt = mybir.dt
    F32, F32R = dt.float32, dt.float32r
    AF = mybir.ActivationFunctionType
    ALU = mybir.AluOpType

    nc = bacc.Bacc("TRN2", target_bir_lowering=False, debug=False, num_devices=NCORES)

    xT = nc.dram_tensor("xT", [D, S], F32R, kind="ExternalInput")
    xT_own = nc.dram_tensor("xT_own", [D, CH], F32R, kind="ExternalInput")
    cosT = nc.dram_tensor("cosT", [HF, S], F32, kind="ExternalInput")
    sinT = nc.dram_tensor("sinT", [HF, S], F32, kind="ExternalInput")
    cosT_own = nc.dram_tensor("cosT_own", [HF, CH], F32, kind="ExternalInput")
    sinT_own = nc.dram_tensor("sinT_own", [HF, CH], F32, kind="ExternalInput")
    # weights pre-tiled host-side: [outblk, 128 part(din%128), din//128, outcols]
    wq = nc.dram_tensor("wq", [H, P, ND, HD], F32R, kind="ExternalInput")
    wk = nc.dram_tensor("wk", [H, P, ND, HD], F32R, kind="ExternalInput")
    wv = nc.dram_tensor("wv", [4, P, ND, 4 * HD], F32R, kind="ExternalInput")
    wo = nc.dram_tensor("wo", [ND, P, ND, P], F32R, kind="ExternalInput")
    w1 = nc.dram_tensor("w1", [NHID, P, ND, P], F32R, kind="ExternalInput")
    w3 = nc.dram_tensor("w3", [NHID, P, ND, P], F32R, kind="ExternalInput")
    w2 = nc.dram_tensor("w2", [ND, P, NHID, P], F32R, kind="ExternalInput")
    yT = nc.dram_tensor("yT", [D, CH], F32, kind="ExternalOutput")

    def rope_evict(rope_p, ps_in, out_ap, cos1, sin1):
        t1 = rope_p.tile([HF, cos1.shape[-1]], F32, tag="t1", name="t1")
        t2 = rope_p.tile([HF, cos1.shape[-1]], F32, tag="t2", name="t2")
        t3 = rope_p.tile([HF, cos1.shape[-1]], F32, tag="t1", name="t3")
        t4 = rope_p.tile([HF, cos1.shape[-1]], F32, tag="t2", name="t4")
        a = ps_in[0:HF, :]
        b = ps_in[HF:HD, :]
        nc.vector.tensor_tensor(t1[:], a, cos1, ALU.mult)
        nc.vector.tensor_tensor(t2[:], b, sin1, ALU.mult)
        nc.vector.tensor_tensor(t3[:], a, sin1, ALU.mult)
        nc.vector.tensor_tensor(t4[:], b, cos1, ALU.mult)
        nc.vector.tensor_tensor(out_ap[0:HF, :], t1[:], t2[:], ALU.subtract)
        nc.vector.tensor_tensor(out_ap[HF:HD, :], t3[:], t4[:], ALU.add)

    def rms_row(pool, pspool, src3d, n, ones_c, eps_c):
        """1/sqrt(mean_d(src^2)+eps) along partitions+tiles -> [1, n] f32 tile."""
        ps_ms = pspool.tile([1, n], F32, tag="msrow", name="ps_ms")
        for d in range(ND):
            sq = pool.tile([P, n], F32R, tag="sq", name="sq")
            nc.scalar.activation(sq[:], src3d[:, d, :], AF.Square)
            nc.tensor.matmul(ps_ms[:], ones_c[:, 0:1], sq[:],
                             start=(d == 0), stop=(d == ND - 1))
        rms = pool.tile([1, n], F32, tag="rms", name="rms")
        nc.scalar.activation(rms[:], ps_ms[:], AF.Sqrt, bias=eps_c[0:1, :], scale=1.0 / D)
        srow = pool.tile([1, n], F32, tag="srow", name="srow")
        nc.vector.reciprocal(srow[:], rms[:])
        return srow

    with tile.TileContext(nc) as tc:
        loop_cm = tc.For_i(0, loop_iters, 1) if loop_iters else None
        if loop_cm is not None:
            loop_cm.__enter__()
        dram_cm = tc.tile_pool(name="dram", bufs=1, space="DRAM")
        dram = dram_cm.__enter__()
        consts_cm = tc.tile_pool(name="consts", bufs=1)
        consts = consts_cm.__enter__()
        own_cm = tc.tile_pool(name="own", bufs=1)
        own = own_cm.__enter__()

        KT_d = dram.tile([H, HD, S], F32R)
        V_d = dram.tile([S, D], F32R)

        ones_f = consts.tile([P, 2], F32)
        nc.vector.memset(ones_f[:], 1.0)
        ones_c = consts.tile([P, 2], F32R)
        nc.vector.tensor_copy(ones_c[:], ones_f[:])
        eps_c = consts.tile([P, 1], F32)
        nc.vector.memset(eps_c[:], EPS)

        xo = own.tile([P, ND, CH], F32R)
        nc.scalar.dma_start(xo[:], xT_own.ap().rearrange("(a p) t -> p a t", p=P))

        # ================= Phase A: full-seq s, K^T -> HBM, V -> HBM =================
        with tc.tile_pool(name="xa", bufs=1) as xa_p, \
             tc.tile_pool(name="sqp", bufs=2) as sq_p, \
             tc.tile_pool(name="wstr", bufs=2) as wstr, \
             tc.tile_pool(name="wvg", bufs=1) as wvg_p, \
             tc.tile_pool(name="rope", bufs=2) as rope_p, \
             tc.tile_pool(name="ev", bufs=3) as ev_p, \
             tc.tile_pool(name="sc", bufs=1) as sc_p, \
             tc.tile_pool(name="psk", bufs=4, space="PSUM") as psk, \
             tc.tile_pool(name="psv", bufs=2, space="PSUM") as psv, \
             tc.tile_pool(name="pss", bufs=1, space="PSUM") as pss:

            for half in range(2):
                t0 = half * (S // 2)
                HS = S // 2
                xa = xa_p.tile([P, ND, HS], F32R, tag="xa", name="xa")
                nc.scalar.dma_start(
                    xa[:], xT.ap()[:, t0:t0 + HS].rearrange("(a p) t -> p a t", p=P))

                # --- per-512-chunk s_row -> cos1/sin1 (norm scale folded) ---
                cos1 = sc_p.tile([HF, HS], F32, tag="cos1", name="cos1")
                sin1 = sc_p.tile([HF, HS], F32, tag="sin1", name="sin1")
                for c in range(HS // CH):
                    sl = slice(c * CH, (c + 1) * CH)
                    srow = rms_row(sq_p, pss, xa[:, :, sl], CH, ones_c, eps_c)
                    sb = sc_p.tile([HF, CH], F32, tag="sb", name="sb")
                    nc.gpsimd.partition_broadcast(sb[:], srow[:])
                    ct = sq_p.tile([HF, CH], F32, tag="ct", name="ct")
                    nc.scalar.dma_start(ct[:], cosT[:, t0 + c * CH:t0 + (c + 1) * CH])
                    st = sq_p.tile([HF, CH], F32, tag="st", name="st")
                    nc.scalar.dma_start(st[:], sinT[:, t0 + c * CH:t0 + (c + 1) * CH])
                    nc.vector.tensor_tensor(cos1[:, sl], ct[:], sb[:], ALU.mult)
                    nc.vector.tensor_tensor(sin1[:, sl], st[:], sb[:], ALU.mult)

                # --- s_col (per-token 1/rms as column) for V scaling ---
                scol_i = sc_p.tile([P, NT // 2], F32, tag="scoli", name="scol_i")
                scol = sc_p.tile([P, NT // 2], F32, tag="scol", name="scol")
                for tt in range(NT // 2):
                    ps_mc = pss.tile([P, 2], F32, tag="mscol", name="ps_mc")
                    for d in range(ND):
                        sqc = sq_p.tile([P, P], F32R, tag="sqc", name="sqc")
                        nc.scalar.activation(sqc[:], xa[:, d, tt * P:(tt + 1) * P], AF.Square)
                        nc.tensor.matmul(ps_mc[:], sqc[:], ones_c[:],
                                         start=(d == 0), stop=(d == ND - 1))
                    nc.scalar.activation(scol[:, tt:tt + 1], ps_mc[:, 0:1], AF.Sqrt,
                                         bias=eps_c[:], scale=1.0 / D)
                nc.vector.reciprocal(scol_i[:], scol[:])

                # --- K^T projection + RoPE -> KT_d (weights loaded once, 2 chunks) ---
                for h in range(H):
                    wkh = wstr.tile([P, ND, HD], F32R, tag="w", name="wkh")
                    nc.sync.dma_start(wkh[:], wk.ap()[h])
                    ps_k0 = psk.tile([P, CH], F32, tag="proj", name="ps_k0")
                    ps_k1 = psk.tile([P, CH], F32, tag="proj", name="ps_k1")
                    for d in range(ND):
                        nc.tensor.matmul(ps_k0[:], wkh[:, d, :], xa[:, d, 0:CH],
                                         start=(d == 0), stop=(d == ND - 1))
                        nc.tensor.matmul(ps_k1[:], wkh[:, d, :], xa[:, d, CH:2 * CH],
                                         start=(d == 0), stop=(d == ND - 1))
                    for c, ps_k in ((0, ps_k0), (1, ps_k1)):
                        sl = slice(c * CH, (c + 1) * CH)
                        kt = ev_p.tile([P, CH], F32R, tag="kt", name="kt")
                        rope_evict(rope_p, ps_k, kt[:], cos1[:, sl], sin1[:, sl])
                        nc.gpsimd.dma_start(KT_d[h, :, t0 + c * CH:t0 + (c + 1) * CH], kt[:])

                # --- V projection (token-major, s-scaled) -> V_d ---
                for g in range(4):
                    wvg = wvg_p.tile([P, ND, 4 * HD], F32R, tag="wvg", name="wvg")
                    nc.sync.dma_start(wvg[:], wv.ap()[g])
                    for tt in range(NT // 2):
                        ps_v = psv.tile([P, 4 * HD], F32, tag="vproj", name="ps_v")
                        for d in range(ND):
                            nc.tensor.matmul(ps_v[:], xa[:, d, tt * P:(tt + 1) * P],
                                             wvg[:, d, :],
                                             start=(d == 0), stop=(d == ND - 1))
                        vt = ev_p.tile([P, 4 * HD], F32R, tag="vt", name="vt")
                        nc.vector.tensor_scalar_mul(vt[:], ps_v[:], scol_i[:, tt:tt + 1])
                        nc.gpsimd.dma_start(
                            V_d[t0 + tt * P:t0 + (tt + 1) * P, g * 4 * HD:(g + 1) * 4 * HD],
                            vt[:])

        # ================= Phase B: Q proj + attention =================
        attnp_cm = tc.tile_pool(name="attnp", bufs=1)
        attnp = attnp_cm.__enter__()
        attnT = attnp.tile([P, H, CH], F32R)

        with tc.tile_pool(name="qpool", bufs=1) as qpool, \
             tc.tile_pool(name="sqo", bufs=2) as sqo_p, \
             tc.tile_pool(name="wstrq", bufs=2) as wstrq, \
             tc.tile_pool(name="ropeq", bufs=2) as ropeq, \
             tc.tile_pool(name="kv", bufs=2) as kv_p, \
             tc.tile_pool(name="et", bufs=4) as et_p, \
             tc.tile_pool(name="bi", bufs=2) as bi_p, \
             tc.tile_pool(name="psq", bufs=2, space="PSUM") as psq, \
             tc.tile_pool(name="ps_s", bufs=2, space="PSUM") as ps_s, \
             tc.tile_pool(name="ps_o", bufs=2, space="PSUM") as ps_o, \
             tc.tile_pool(name="ps_d", bufs=1, space="PSUM") as ps_d, \
             tc.tile_pool(name="pssq", bufs=1, space="PSUM") as pssq:

            QTr = qpool.tile([P, H, CH], F32R)
            # own-chunk rms scale + rope tables
            srow_o = rms_row(sqo_p, pssq, xo[:], CH, ones_c, eps_c)
            sbo = sqo_p.tile([HF, CH], F32, tag="sb", name="sbo")
            nc.gpsimd.partition_broadcast(sbo[:], srow_o[:])
            cos1o = qpool.tile([HF, CH], F32)
            sin1o = qpool.tile([HF, CH], F32)
            cto = sqo_p.tile([HF, CH], F32, tag="ct", name="cto")
            nc.scalar.dma_start(cto[:], cosT_own[:])
            sto = sqo_p.tile([HF, CH], F32, tag="st", name="sto")
            nc.scalar.dma_start(sto[:], sinT_own[:])
            nc.vector.tensor_tensor(cos1o[:], cto[:], sbo[:], ALU.mult)
            nc.vector.tensor_tensor(sin1o[:], sto[:], sbo[:], ALU.mult)

            for h in range(H):
                wqh = wstrq.tile([P, ND, HD], F32R, tag="w", name="wqh")
                nc.sync.dma_start(wqh[:], wq.ap()[h])
                ps_q = psq.tile([P, CH], F32, tag="qproj", name="ps_q")
                for d in range(ND):
                    nc.tensor.matmul(ps_q[:], wqh[:, d, :], xo[:, d, :],
                                     start=(d == 0), stop=(d == ND - 1))
                rope_evict(ropeq, ps_q, QTr[:, h, :], cos1o[:], sin1o[:])

            for h in range(H):
                ktr = kv_p.tile([P, S], F32R, tag="ktr", name="ktr")
                nc.scalar.dma_start(ktr[:], KT_d[h, :, :])
                vh = kv_p.tile([P, NT, HD], F32R, tag="vh", name="vh")
                nc.scalar.dma_start(
                    vh[:], V_d[:].rearrange("(a p) d -> p a d", p=P)[:, :, h * HD:(h + 1) * HD])
                po = ps_o.tile([P, CH], F32, tag="o", name="po")
                pd = ps_d.tile([1, CH], F32, tag="d", name="pd")
                for kt in range(NT):
                    ps_sc = ps_s.tile([P, CH], F32, tag="s", name="ps_sc")
                    nc.tensor.matmul(ps_sc[:], ktr[:, kt * P:(kt + 1) * P], QTr[:, h, :],
                                     start=True, stop=True)
                    e = et_p.tile([P, CH], F32R, tag="e", name="e")
                    nc.scalar.activation(e[:], ps_sc[:], AF.Exp, scale=ISQ)
                    nc.tensor.matmul(po[:], vh[:, kt, :], e[:],
                                     start=(kt == 0), stop=(kt == NT - 1))
                    nc.tensor.matmul(pd[:], ones_c[:, 0:1], e[:],
                                     start=(kt == 0), stop=(kt == NT - 1))
                inv = bi_p.tile([1, CH], F32, tag="inv", name="inv")
                nc.vector.reciprocal(inv[:], pd[:])
                binv = bi_p.tile([P, CH], F32, tag="binv", name="binv")
                nc.gpsimd.partition_broadcast(binv[:], inv[:])
                nc.vector.tensor_tensor(attnT[:, h, :], po[:], binv[:], ALU.mult)

        # ================= Phase C: wo + residual -> hT =================
        hp_cm = tc.tile_pool(name="hp", bufs=1, side="right")
        hp = hp_cm.__enter__()
        hT = hp.tile([P, ND, CH], F32)

        with tc.tile_pool(name="wstr2", bufs=2) as wstr2, \
             tc.tile_pool(name="ps_w", bufs=2, space="PSUM") as ps_w:
            for j in range(ND):
                woj = wstr2.tile([P, ND, P], F32R, tag="w", name="woj")
                nc.sync.dma_start(woj[:], wo.ap()[j])
                ps_h = ps_w.tile([P, CH], F32, tag="wo", name="ps_h")
                for d in range(ND):
                    nc.tensor.matmul(ps_h[:], woj[:, d, :], attnT[:, d, :],
                                     start=(d == 0), stop=(d == ND - 1))
                nc.vector.tensor_tensor(hT[:, j, :], ps_h[:], xo[:, j, :], ALU.add)

        attnp_cm.__exit__(None, None, None)
        own_cm.__exit__(None, None, None)

        # ================= Phase D: FFN =================
        with tc.tile_pool(name="swp", bufs=1) as sw_p:
            swt = sw_p.tile([P, NHID, CH], F32R)
            with tc.tile_pool(name="fp", bufs=1) as fp, \
                 tc.tile_pool(name="fstr13", bufs=2) as fstr13, \
                 tc.tile_pool(name="fev1", bufs=2) as fev1, \
                 tc.tile_pool(name="fsc", bufs=2) as fsc, \
                 tc.tile_pool(name="ps_u", bufs=2, space="PSUM") as ps_u, \
                 tc.tile_pool(name="ps_g", bufs=2, space="PSUM") as ps_g, \
                 tc.tile_pool(name="ps_n", bufs=1, space="PSUM") as ps_n:

                s2 = rms_row(fsc, ps_n, hT[:], CH, ones_c, eps_c)
                bs2 = fsc.tile([P, CH], F32, tag="bs2", name="bs2")
                nc.gpsimd.partition_broadcast(bs2[:], s2[:])
                hnT = fp.tile([P, ND, CH], F32R)
                for d in range(ND):
                    nc.vector.tensor_tensor(hnT[:, d, :], hT[:, d, :], bs2[:], ALU.mult)

                for k in range(NHID):
                    w1k = fstr13.tile([P, ND, P], F32R, tag="w1", name="w1k")
                    nc.sync.dma_start(w1k[:], w1.ap()[k])
                    ps_uu = ps_u.tile([P, CH], F32, tag="u", name="ps_uu")
                    for d in range(ND):
                        nc.tensor.matmul(ps_uu[:], w1k[:, d, :], hnT[:, d, :],
                                         start=(d == 0), stop=(d == ND - 1))
                    su = fev1.tile([P, CH], F32R, tag="su", name="su")
                    nc.scalar.activation(su[:], ps_uu[:], AF.Silu)
                    w3k = fstr13.tile([P, ND, P], F32R, tag="w3", name="w3k")
                    nc.sync.dma_start(w3k[:], w3.ap()[k])
                    ps_gg = ps_g.tile([P, CH], F32, tag="g", name="ps_gg")
                    for d in range(ND):
                        nc.tensor.matmul(ps_gg[:], w3k[:, d, :], hnT[:, d, :],
                                         start=(d == 0), stop=(d == ND - 1))
                    nc.vector.tensor_tensor(swt[:, k, :], ps_gg[:], su[:], ALU.mult)

            with tc.tile_pool(name="fstr2", bufs=2) as fstr2, \
                 tc.tile_pool(name="fev2", bufs=2) as fev2, \
                 tc.tile_pool(name="ps_y", bufs=2, space="PSUM") as ps_y:
                for j in range(ND):
                    w2j = fstr2.tile([P, NHID, P], F32R, tag="w2", name="w2j")
                    nc.sync.dma_start(w2j[:], w2.ap()[j])
                    ps_yy = ps_y.tile([P, CH], F32, tag="y", name="ps_yy")
                    for k in range(NHID):
                        nc.tensor.matmul(ps_yy[:], w2j[:, k, :], swt[:, k, :],
                                         start=(k == 0), stop=(k == NHID - 1))
                    yt = fev2.tile([P, CH], F32, tag="yt", name="yt")
                    nc.vector.tensor_tensor(yt[:], ps_yy[:], hT[:, j, :], ALU.add)
                    nc.sync.dma_start(yT[j * P:(j + 1) * P, :], yt[:])

        hp_cm.__exit__(None, None, None)
        consts_cm.__exit__(None, None, None)
        dram_cm.__exit__(None, None, None)
        if loop_cm is not None:
            loop_cm.__exit__(None, None, None)

    nc.compile()
    return nc


class _Runner:
    def __init__(self, nc, n_cores=NCORES):
        import jax
        from jax.sharding import Mesh, PartitionSpec
        from jax.experimental.shard_map import shard_map
        from concourse import mybir
        from concourse.bass2jax import _bass_exec_p, install_neuronx_cc_hook, partition_id_tensor

        install_neuronx_cc_hook()
        self.nc = nc
        self.n_cores = n_cores
        partition_name = nc.partition_id_tensor.name if nc.partition_id_tensor else None
        in_names, out_names, out_avals = [], [], []
        for alloc in nc.m.functions[0].allocations:
            if not isinstance(alloc, mybir.MemoryLocationSet):
                continue
            name = alloc.memorylocations[0].name
            if alloc.kind == "ExternalInput":
                if name != partition_name and name != (nc.dbg_addr.name if nc.dbg_addr else None):
                    in_names.append(name)
            elif alloc.kind == "ExternalOutput":
                out_names.append(name)
                out_avals.append(jax.core.ShapedArray(tuple(alloc.tensor_shape), mybir.dt.np(alloc.dtype)))
        self.in_names, self.out_names, self.out_avals = in_names, out_names, out_avals
        has_dbg = nc.dbg_addr is not None
        all_in = tuple(in_names + out_names
                       + ([nc.dbg_addr.name] if has_dbg else [])
                       + ([partition_name] if partition_name else []))

        def _body(*args):
            import jax.numpy as jnp
            operands = list(args)
            if has_dbg:
                operands.append(jnp.zeros((1, 2), jnp.uint32))
            if partition_name is not None:
                operands.append(partition_id_tensor())
            outs = _bass_exec_p.bind(
                *operands,
                out_avals=tuple(out_avals),
                in_names=all_in,
                out_names=tuple(out_names),
                lowering_input_output_aliases=(),
                sim_require_finite=False,
                sim_require_nnan=False,
                nc=nc,
            )
            return tuple(outs)

        devices = jax.devices()[:n_cores]
        self.mesh = Mesh(np.asarray(devices), ("core",))
        n_params = len(in_names)
        in_specs = (PartitionSpec("core"),) * (n_params + len(out_names))
        out_specs = (PartitionSpec("core"),) * len(out_names)
        self.fn = jax.jit(
            shard_map(_body, mesh=self.mesh, in_specs=in_specs, out_specs=out_specs,
                      check_rep=False),
            keep_unused=True,
        )

    def stage(self, in_maps):
        import jax
        from jax.sharding import PartitionSpec
        n = self.n_cores
        concat_in = [
            np.concatenate([np.asarray(in_maps[c][name]) for c in range(n)], axis=0)
            for name in self.in_names
        ]
        concat_zeros = [np.zeros((n * a.shape[0], *a.shape[1:]), a.dtype) for a in self.out_avals]
        sharding = jax.sharding.NamedSharding(self.mesh, PartitionSpec("core"))
        staged = [jax.device_put(x, sharding) for x in concat_in + concat_zeros]
        for x in staged:
            x.block_until_ready()
        return staged

    def run_staged(self, staged):
        import jax
        out = self.fn(*staged)
        jax.block_until_ready(out)
        return out

    def run(self, in_maps):
        out_arrs = self.run_staged(self.stage(in_maps))
        n = self.n_cores
        return [
            {name: np.asarray(out_arrs[i]).reshape(n, *self.out_avals[i].shape)[c]
             for i, name in enumerate(self.out_names)}
            for c in range(n)
        ]


def _perm_pairs():
    p = np.arange(D).reshape(H, HD // 2, 2)
    return np.concatenate([p[..., 0], p[..., 1]], axis=-1).reshape(-1)


def make_in_maps(x, freqs_cos, freqs_sin, wq, wk, wv, wo, w1, w2, w3,
                 attn_norm_w, ffn_norm_w):
    f32 = np.float32
    x = np.asarray(x, f32)
    cos = np.ascontiguousarray(np.asarray(freqs_cos, f32).reshape(S, HD // 2).T)
    sin = np.ascontiguousarray(np.asarray(freqs_sin, f32).reshape(S, HD // 2).T)
    perm = _perm_pairs()
    anw = np.asarray(attn_norm_w, f32)
    fnw = np.asarray(ffn_norm_w, f32)
    def blk(w, nblk, ncols):
        # [din, dout] -> [nblk, 128, din//128, ncols]
        din = w.shape[0]
        return np.ascontiguousarray(
            w.reshape(din // P, P, nblk, ncols).transpose(2, 1, 0, 3))

    wq_p = blk((np.asarray(wq, f32) * anw[:, None])[:, perm], H, HD)
    wk_p = blk((np.asarray(wk, f32) * anw[:, None])[:, perm], H, HD)
    wv_f = blk(np.asarray(wv, f32) * anw[:, None], 4, 4 * HD)
    wo_f = blk(np.asarray(wo, f32), ND, P)
    w1_f = blk(np.asarray(w1, f32) * fnw[:, None], NHID, P)
    w3_f = blk(np.asarray(w3, f32) * fnw[:, None], NHID, P)
    w2_f = blk(np.asarray(w2, f32), ND, P)
    xT = [np.ascontiguousarray(x[b].T) for b in range(B)]
    in_maps = []
    for core in range(NCORES):
        b, c = divmod(core, 4)
        in_maps.append({
            "xT": xT[b],
            "xT_own": np.ascontiguousarray(xT[b][:, c * CH:(c + 1) * CH]),
            "cosT": cos, "sinT": sin,
            "cosT_own": np.ascontiguousarray(cos[:, c * CH:(c + 1) * CH]),
            "sinT_own": np.ascontiguousarray(sin[:, c * CH:(c + 1) * CH]),
            "wq": wq_p, "wk": wk_p, "wv": wv_f, "wo": wo_f,
            "w1": w1_f, "w3": w3_f, "w2": w2_f,
        })
    return in_maps


def get_runner():
    if "runner" not in _cache:
        _patch_ldw_opt()
        nc = build_nc()
        _cache["runner"] = _Runner(nc, NCORES)
    return _cache["runner"]


def kernel(**inputs) -> np.ndarray:
    r = get_runner()
    in_maps = make_in_maps(**inputs)
    res = r.run(in_maps)
    y = np.empty((B, S, D), np.float32)
    for core in range(NCORES):
        b, c = divmod(core, 4)
        y[b, c * CH:(c + 1) * CH, :] = res[core]["yT"].T
    return y
